# revision 24
# baseline (speedup 1.0000x reference)
"""Cross-attention kernel for Trainium2, batch-data-parallel over 8 NeuronCores.

Reference computation (per batch element b):
    q = x Wq + bq ; k = c Wk + bk ; v = c Wv + bv          (DIM=1024)
    per head h (16 heads, d=64):
        S = (q_h k_h^T) * d^-0.5 ; P = softmax(S, axis=-1) ; o_h = P v_h
    out = concat_h(o_h) Wo + bo

Layout strategy (per core, one batch element):
    Host passes x^T and c^T (bf16) so every matmul contraction dim sits on
    SBUF partitions.  QT=[dout,n], KT=[dout,m] are produced directly by
    lhsT=W (stationary), rhs=x^T.  Attention scores are computed transposed,
    ST=[m,n], which makes P@V a plain accumulation with stationary V[m,d].
    V is augmented with a ones column so the softmax denominator falls out of
    the same matmul (row d of the PSUM tile).  Softmax skips the max-subtract
    (scores are ~N(0,1), exp cannot overflow).  The final projection consumes
    OT=[hd,n] as the stationary operand, yielding out=[n,dout] directly.

    The two heads of a pair occupy disjoint 64-partition halves, so their
    score matmuls (K=64) are issued as an explicit 64x128 row-tiled pair
    (tile_position (0,0)/(64,0)) for PE subarray concurrency.  The softmax
    tail (ln/exp reciprocal on ACT, PE broadcast, normalize multiply) is
    deferred one block so it never head-of-line blocks the PE.  The final
    projection for n<512 is emitted before the last block's tail flush so it
    overlaps the tail drain; output is stored bf16 (host upcasts) to halve
    the non-overlappable final DMA.
"""

import os

import numpy as np
import ml_dtypes

import concourse.bass as bass
import concourse.bacc as bacc
import concourse.mybir as mybir
import concourse.tile as tile

B = 8
SEQ = 1024          # N == M == 1024
DIM = 1024
H = 16
HD = DIM // H       # 64
SCALE = HD ** -0.5
P = 128
NCH = DIM // P      # 8
HW = HD + 1         # head width in the augmented V (64 values + ones col)

BF16 = mybir.dt.bfloat16
F32 = mybir.dt.float32
NPBF16 = ml_dtypes.bfloat16
EXP = mybir.ActivationFunctionType.Exp
LOG = mybir.ActivationFunctionType.Ln


class _Bacc(bacc.Bacc):
    def insert_act_table_loads(self):
        # Prefer natural_log_exp_and_others (has BOTH Exp and Ln) so the
        # softmax exp and the exp(-ln) reciprocal share one table set —
        # otherwise the pass alternates sets and pays ~2.7us per switch.
        from concourse.hw_specs import get_activation_tables
        import bass_rust as _br
        tables = list(get_activation_tables(self.m.arch).items())
        canon = [k for k, _ in tables]
        tables.sort(key=lambda kv: kv[0] != "natural_log_exp_and_others")
        _br.insert_act_table_loads(self, tables)
        # The rust pass numbers sets by position in the list it was given;
        # walrus maps act_func_set_id against act_info.json's canonical
        # order.  Re-point every emitted load (all natural_log_exp here) at
        # the canonical index.
        want = canon.index("natural_log_exp_and_others")
        for f in self.m.functions:
            for b in f.blocks:
                for i in b.instructions:
                    if isinstance(i, mybir.InstLoadActFuncSet):
                        i.act_func_set_id = want


def build_nc() -> bass.Bass:
    # Bacc (not plain Bass): its compile() splits multi-sem sync waits into
    # event semaphores (walrus only encodes 1 wait per instruction) and
    # auto-inserts GPSIMD library / ACT table loads.
    nc = _Bacc("TRN2")

    xt_d = nc.declare_dram_parameter("xt", [NCH, P, SEQ], BF16, isOutput=False)
    ct_d = nc.declare_dram_parameter("ct", [NCH, P, SEQ], BF16, isOutput=False)
    wq_d = nc.declare_dram_parameter("wq", [NCH, P, DIM], BF16, isOutput=False)
    wk_d = nc.declare_dram_parameter("wk", [NCH, P, DIM], BF16, isOutput=False)
    wv_d = nc.declare_dram_parameter("wv", [NCH, P, DIM], BF16, isOutput=False)
    wo_d = nc.declare_dram_parameter("wo", [NCH, P, DIM], BF16, isOutput=False)
    bq_d = nc.declare_dram_parameter("bq", [P, NCH], F32, isOutput=False)
    bk_d = nc.declare_dram_parameter("bk", [P, NCH], F32, isOutput=False)
    bv_d = nc.declare_dram_parameter("bv", [DIM], F32, isOutput=False)
    bo_d = nc.declare_dram_parameter("bo", [DIM], F32, isOutput=False)
    # bf16 output halves the non-overlappable final DMA; host upcasts.
    out_d = nc.declare_dram_parameter("out", [SEQ, DIM], BF16, isOutput=True)

    with tile.TileContext(nc) as tc:
        with (
            tc.tile_pool(name="big", bufs=1) as big,
            tc.tile_pool(name="wts", bufs=1) as wts,
            tc.tile_pool(name="ctot", bufs=1) as ctot,
            tc.tile_pool(name="expp", bufs=3) as expp,
            tc.tile_pool(name="smal", bufs=2) as smal,
            tc.tile_pool(name="outp", bufs=3) as outp,
            tc.tile_pool(name="ppr", bufs=2, space="PSUM") as ppr,
            tc.tile_pool(name="pot", bufs=2, space="PSUM") as pot,
            tc.tile_pool(name="pst", bufs=2, space="PSUM") as pst,
        ):
            # ---- persistent SBUF tensors ----
            # wv/wo rotate through one slot: wv is dead once the V projection
            # ends, and wo is only read by the final projection.
            ct_sb = ctot.tile([P, NCH, SEQ], BF16, tag="ctot", name="ct")
            wv_sb = wts.tile([P, NCH, DIM], BF16, tag="w", name="wv")
            wk_sb = big.tile([P, NCH, DIM], BF16, tag="wk")
            wq_sb = big.tile([P, NCH, DIM], BF16, tag="wq")
            wo_sb = wts.tile([P, NCH, DIM], BF16, tag="w", name="wo")
            xt_sb = big.tile([P, NCH, SEQ], BF16, tag="xt")
            kt_sb = big.tile([P, NCH, SEQ], BF16, tag="kt")
            qt_sb = big.tile([P, NCH, SEQ], BF16, tag="qt")
            vaug_sb = big.tile([P, NCH, H * HW], BF16, tag="vaug")
            bq_sb = big.tile([P, NCH], F32, tag="bq")
            bk_sb = big.tile([P, NCH], F32, tag="bk")
            bvb_sb = big.tile([P, DIM], F32, tag="bvb")
            bob_sb = big.tile([P, DIM], F32, tag="bob")

            # ---- input DMAs, ordered by first use (K projection first) ----
            for j in range(NCH):
                nc.sync.dma_start(out=ct_sb[:, j, 0:512], in_=ct_d[j][:, 0:512])
            nc.sync.dma_start(out=bk_sb, in_=bk_d[:, :])
            for j in range(NCH):
                nc.sync.dma_start(out=wk_sb[:, j, :], in_=wk_d[j])
            for j in range(NCH):
                nc.sync.dma_start(out=ct_sb[:, j, 512:1024], in_=ct_d[j][:, 512:1024])
            for j in range(NCH):
                nc.sync.dma_start(out=xt_sb[:, j, :], in_=xt_d[j])
            for j in range(NCH):
                nc.sync.dma_start(out=wq_sb[:, j, :], in_=wq_d[j])
            nc.sync.dma_start(out=bq_sb, in_=bq_d[:, :])
            for j in range(NCH):
                nc.sync.dma_start(out=wv_sb[:, j, :], in_=wv_d[j])
            for (dst, src) in ((bvb_sb, bv_d), (bob_sb, bo_d)):
                ap = src[:]
                bcast = bass.AP(tensor=ap.tensor, offset=ap.offset,
                                ap=[[0, P]] + ap.ap)
                nc.sync.dma_start(out=dst, in_=bcast)
            # wo reuses wv's slot: its DMA waits until the V phase finishes;
            # wo itself is only read by the final projection, much later.
            for j in range(NCH):
                nc.sync.dma_start(out=wo_sb[:, j, :], in_=wo_d[j])

            vaug4 = vaug_sb.rearrange("p j (h e) -> p j h e", e=HW)
            nc.vector.memset(vaug4[:, :, :, HD:HW], 1.0)
            ones_sb = big.tile([1, HD], BF16, tag="ones")
            nc.vector.memset(ones_sb, 1.0)

            # ---- V = c Wv + bv, written per-head into the augmented layout.
            # Emitted in dh halves: heads 0-7 (dh=0) are needed by the first
            # attention pair, heads 8-15 (dh=1) only from pair 4 on, so the
            # halves are slotted around pair 0 to start ACT early. ----
            def v_phase(dh):
                for mm in range(NCH):
                    pv = ppr.tile([P, 512], F32, tag="ppr", name="pv")
                    for k in range(NCH):
                        nc.tensor.matmul(
                            pv,
                            lhsT=ct_sb[:, k, mm * P:(mm + 1) * P],
                            rhs=wv_sb[:, k, dh * 512:(dh + 1) * 512],
                            start=(k == 0), stop=(k == NCH - 1),
                        )
                    pvv = pv.rearrange("p (h e) -> p h e", e=HD)
                    bvv = bvb_sb[:, dh * 512:(dh + 1) * 512].rearrange(
                        "p (h e) -> p h e", e=HD)
                    nc.vector.tensor_add(
                        vaug4[:, mm, dh * 8:(dh + 1) * 8, 0:HD], pvv, bvv)

            def q_proj(jq):
                for mh in range(2):
                    pq = ppr.tile([P, 512], F32, tag="ppr", name="pq")
                    for k in range(NCH):
                        nc.tensor.matmul(
                            pq,
                            lhsT=wq_sb[:, k, jq * P:(jq + 1) * P],
                            rhs=xt_sb[:, k, mh * 512:(mh + 1) * 512],
                            start=(k == 0), stop=(k == NCH - 1),
                        )
                    nc.vector.tensor_scalar_add(
                        qt_sb[:, jq, mh * 512:(mh + 1) * 512], pq,
                        bq_sb[:, jq:jq + 1])

            def k_proj(jq, mh):
                pk = ppr.tile([P, 512], F32, tag="ppr", name="pk")
                for k in range(NCH):
                    nc.tensor.matmul(
                        pk,
                        lhsT=wk_sb[:, k, jq * P:(jq + 1) * P],
                        rhs=ct_sb[:, k, mh * 512:(mh + 1) * 512],
                        start=(k == 0), stop=(k == NCH - 1),
                    )
                nc.vector.tensor_scalar_add(
                    kt_sb[:, jq, mh * 512:(mh + 1) * 512], pk,
                    bk_sb[:, jq:jq + 1])

            # ---- fused attention ----
            ot_sb = ctot.tile([P, NCH, SEQ], BF16, tag="ctot", name="ot")

            # Deferred normalize tail: a block's ln/exp + PE broadcast +
            # normalize multiply are emitted only after the NEXT block's
            # matmuls, so the ACT latency never head-of-line blocks the PE.
            pend = []

            def flush_tail():
                for (cs_, otsA_, otsB_, jh_, nsl_) in pend:
                    # 1/colsum as exp(-ln(colsum)) on ACT (Ln and Exp share
                    # one table set).  DVE reciprocal was tried here and is
                    # far slower (multi-pass Newton on a 1-partition tile).
                    rl = smal.tile([1, 2, 512], F32, tag="rl", name="rl")
                    nc.scalar.activation(rl, cs_, LOG)
                    rc = smal.tile([1, 2, 512], BF16, tag="rc", name="rc")
                    nc.scalar.activation(rc, rl, EXP, scale=-1.0)
                    # DVE may read only one PSUM operand, so the P@V rows
                    # were copied to SBUF (ots) at block time; rbp stays in
                    # PSUM.
                    rbp = ppr.tile([P, 512], F32, tag="ppr", name="rbp")
                    nc.tensor.matmul(rbp[0:HD, :], lhsT=ones_sb,
                                     rhs=rc[:, 0, :], start=True, stop=True,
                                     tile_position=(0, 0))
                    nc.tensor.matmul(rbp[HD:P, :], lhsT=ones_sb,
                                     rhs=rc[:, 1, :], start=True, stop=True,
                                     tile_position=(0, 64))
                    nc.vector.tensor_mul(
                        ot_sb[0:HD, jh_, nsl_], otsA_, rbp[0:HD, :])
                    nc.vector.tensor_mul(
                        ot_sb[HD:P, jh_, nsl_], otsB_, rbp[HD:P, :])
                pend.clear()

            def st_block(jh, nh):
                nsl = slice(nh * 512, (nh + 1) * 512)
                exA = expp.tile([P, NCH, 512], BF16, tag="ex", name="exA")
                exB = expp.tile([P, NCH, 512], BF16, tag="ex", name="exB")
                for mg in range(4):
                    psA = pst.tile([P, 2, 512], F32, tag="pst", name="psA")
                    psB = pst.tile([P, 2, 512], F32, tag="pst", name="psB")
                    for u in range(2):
                        mm = 2 * mg + u
                        msl = slice(mm * P, (mm + 1) * P)
                        nc.tensor.matmul(
                            psA[:, u, :],
                            lhsT=kt_sb[0:HD, jh, msl],
                            rhs=qt_sb[0:HD, jh, nsl],
                            start=True, stop=True,
                            tile_position=(0, 0),
                        )
                        nc.tensor.matmul(
                            psB[:, u, :],
                            lhsT=kt_sb[HD:P, jh, msl],
                            rhs=qt_sb[HD:P, jh, nsl],
                            start=True, stop=True,
                            tile_position=(64, 0),
                        )
                    nc.scalar.activation(exA[:, 2 * mg:2 * mg + 2, :], psA, EXP)
                    nc.scalar.activation(exB[:, 2 * mg:2 * mg + 2, :], psB, EXP)
                return exA, exB, nsl

            def pv_block(jh, blk):
                exA, exB, nsl = blk
                poA = pot.tile([HD + 1, 512], F32, tag="pot", name="poA")
                for mm in range(NCH):
                    nc.tensor.matmul(
                        poA,
                        lhsT=vaug_sb[:, mm, (2 * jh) * HW:(2 * jh + 1) * HW],
                        rhs=exA[:, mm, :],
                        start=(mm == 0), stop=(mm == NCH - 1),
                    )
                poB = pot.tile([HD + 1, 512], F32, tag="pot", name="poB")
                for mm in range(NCH):
                    nc.tensor.matmul(
                        poB,
                        lhsT=vaug_sb[:, mm, (2 * jh + 1) * HW:(2 * jh + 2) * HW],
                        rhs=exB[:, mm, :],
                        start=(mm == 0), stop=(mm == NCH - 1),
                    )
                cs = smal.tile([1, 2, 512], F32, tag="cs", name="cs")
                nc.vector.tensor_copy(cs[:, 0, :], poA[HD:HD + 1, :])
                nc.vector.tensor_copy(cs[:, 1, :], poB[HD:HD + 1, :])
                otsA = smal.tile([HD, 512], BF16, tag="otsA", name="otsA")
                nc.vector.tensor_copy(otsA, poA[0:HD, :])
                otsB = smal.tile([HD, 512], BF16, tag="otsB", name="otsB")
                nc.vector.tensor_copy(otsB, poB[0:HD, :])
                flush_tail()
                pend.append((cs, otsA, otsB, jh, nsl))

            # ---- out = O Wo + bo (emitted in nn-ranges so the first half
            # can overlap the last attention block's softmax tail) ----
            def out_proj(nn0, nn1):
                for nn in range(nn0, nn1):
                    for dh in range(2):
                        pf = ppr.tile([P, 512], F32, tag="ppr", name="pf")
                        for j in range(NCH):
                            nc.tensor.matmul(
                                pf,
                                lhsT=ot_sb[:, j, nn * P:(nn + 1) * P],
                                rhs=wo_sb[:, j, dh * 512:(dh + 1) * 512],
                                start=(j == 0), stop=(j == NCH - 1),
                            )
                        of = outp.tile([P, 512], BF16, tag="of", name="of")
                        nc.vector.tensor_add(
                            of, pf, bob_sb[:, dh * 512:(dh + 1) * 512])
                        nc.sync.dma_start(
                            out=out_d[nn * P:(nn + 1) * P,
                                      dh * 512:(dh + 1) * 512],
                            in_=of)

            # Attention runs as two nh sweeps (all head pairs at n<512, then
            # n>=512) so that once the first sweep's tails flush, the n<512
            # half of the output projection can be spread through the second
            # sweep instead of serializing at the end with ACT idle.
            for mh in range(2):
                for j_k in range(NCH):
                    k_proj(j_k, mh)
            q_proj(0)
            q_proj(1)
            blk = st_block(0, 0)
            v_phase(0)
            pv_block(0, blk)
            q_proj(2)
            for jh in range(1, NCH):
                if jh + 2 < NCH:
                    q_proj(jh + 2)
                blk = st_block(jh, 0)
                if jh == 1:
                    # before the first ot write (ct and ot share a slot)
                    v_phase(1)
                pv_block(jh, blk)
            for jh in range(NCH):
                pv_block(jh, st_block(jh, 1))
                # After pv_block(0, 1) the last nh=0 tail has flushed, so ot
                # rows n<512 are complete; drip that out_proj half here.
                if 1 <= jh <= 4:
                    out_proj(jh - 1, jh)
            flush_tail()
            out_proj(4, NCH)

    nc.compile()
    return nc


_STATE: dict = {}
LAST_EXEC_NS = None
LAST_PROFILE = None


def _prep_in_maps(x, context, Wq, bq, Wk, bk, Wv, bv, Wo, bo):
    def wpack(w, scale=1.0):
        return (np.asarray(w, np.float32) * scale).astype(NPBF16).reshape(
            NCH, P, DIM)

    wq_r = wpack(Wq, SCALE)
    wk_r = wpack(Wk)
    wv_r = wpack(Wv)
    wo_r = wpack(Wo)
    bq_r = np.ascontiguousarray(
        (np.asarray(bq, np.float32) * SCALE).reshape(NCH, P).T)
    bk_r = np.ascontiguousarray(np.asarray(bk, np.float32).reshape(NCH, P).T)
    bv_r = np.asarray(bv, np.float32)
    bo_r = np.asarray(bo, np.float32)

    in_maps = []
    for c in range(B):
        xt_c = np.ascontiguousarray(np.asarray(x[c], np.float32).T).astype(
            NPBF16).reshape(NCH, P, SEQ)
        ct_c = np.ascontiguousarray(np.asarray(context[c], np.float32).T).astype(
            NPBF16).reshape(NCH, P, SEQ)
        in_maps.append({
            "xt": xt_c, "ct": ct_c,
            "wq": wq_r, "wk": wk_r, "wv": wv_r, "wo": wo_r,
            "bq": bq_r, "bk": bk_r, "bv": bv_r, "bo": bo_r,
        })
    return in_maps


def kernel(x, context, Wq, bq, Wk, bk, Wv, bv, Wo, bo):
    global LAST_EXEC_NS, LAST_PROFILE
    from concourse.bass_utils import run_bass_kernel_spmd

    if "nc" not in _STATE:
        _STATE["nc"] = build_nc()
    nc = _STATE["nc"]

    in_maps = _prep_in_maps(x, context, Wq, bq, Wk, bk, Wv, bv, Wo, bo)
    trace = bool(int(os.environ.get("KERNEL_TRACE", "0")))
    kw = {}
    tmpdir = os.environ.get("KERNEL_TMPDIR")
    if tmpdir:
        os.makedirs(tmpdir, exist_ok=True)
        kw["tmpdir"] = tmpdir
    res = run_bass_kernel_spmd(nc, in_maps, list(range(B)), trace=trace, **kw)
    LAST_EXEC_NS = res.exec_time_ns
    LAST_PROFILE = res.profile_json
    out = np.stack([res.results[c]["out"] for c in range(B)], axis=0)
    return out.astype(np.float32)


# revision 27
# speedup vs baseline: 1.0292x; 1.0292x over previous
"""Cross-attention kernel for Trainium2, batch-data-parallel over 8 NeuronCores.

Reference computation (per batch element b):
    q = x Wq + bq ; k = c Wk + bk ; v = c Wv + bv          (DIM=1024)
    per head h (16 heads, d=64):
        S = (q_h k_h^T) * d^-0.5 ; P = softmax(S, axis=-1) ; o_h = P v_h
    out = concat_h(o_h) Wo + bo

Layout strategy (per core, one batch element):
    Host passes x^T and c^T (bf16) so every matmul contraction dim sits on
    SBUF partitions.  QT=[dout,n], KT=[dout,m] are produced directly by
    lhsT=W (stationary), rhs=x^T.  Attention scores are computed transposed,
    ST=[m,n], which makes P@V a plain accumulation with stationary V[m,d].
    V is augmented with a ones column so the softmax denominator falls out of
    the same matmul (row d of the PSUM tile).  Softmax skips the max-subtract
    (scores are ~N(0,1), exp cannot overflow).  The final projection consumes
    OT=[hd,n] as the stationary operand, yielding out=[n,dout] directly.

    The two heads of a pair occupy disjoint 64-partition halves, so their
    score matmuls (K=64) are issued as an explicit 64x128 row-tiled pair
    (tile_position (0,0)/(64,0)) for PE subarray concurrency.  The softmax
    tail (ln/exp reciprocal on ACT, PE broadcast, normalize multiply) is
    deferred one block so it never head-of-line blocks the PE.  The final
    projection for n<512 is emitted before the last block's tail flush so it
    overlaps the tail drain; output is stored bf16 (host upcasts) to halve
    the non-overlappable final DMA.
"""

import os

import numpy as np
import ml_dtypes

import concourse.bass as bass
import concourse.bacc as bacc
import concourse.mybir as mybir
import concourse.tile as tile

B = 8
SEQ = 1024          # N == M == 1024
DIM = 1024
H = 16
HD = DIM // H       # 64
SCALE = HD ** -0.5
P = 128
NCH = DIM // P      # 8
HW = HD + 1         # head width in the augmented V (64 values + ones col)

BF16 = mybir.dt.bfloat16
F32 = mybir.dt.float32
NPBF16 = ml_dtypes.bfloat16
EXP = mybir.ActivationFunctionType.Exp
LOG = mybir.ActivationFunctionType.Ln


class _Bacc(bacc.Bacc):
    def insert_act_table_loads(self):
        # Prefer natural_log_exp_and_others (has BOTH Exp and Ln) so the
        # softmax exp and the exp(-ln) reciprocal share one table set —
        # otherwise the pass alternates sets and pays ~2.7us per switch.
        from concourse.hw_specs import get_activation_tables
        import bass_rust as _br
        tables = list(get_activation_tables(self.m.arch).items())
        canon = [k for k, _ in tables]
        tables.sort(key=lambda kv: kv[0] != "natural_log_exp_and_others")
        _br.insert_act_table_loads(self, tables)
        # The rust pass numbers sets by position in the list it was given;
        # walrus maps act_func_set_id against act_info.json's canonical
        # order.  Re-point every emitted load (all natural_log_exp here) at
        # the canonical index.
        want = canon.index("natural_log_exp_and_others")
        for f in self.m.functions:
            for b in f.blocks:
                for i in b.instructions:
                    if isinstance(i, mybir.InstLoadActFuncSet):
                        i.act_func_set_id = want


def build_nc() -> bass.Bass:
    # Bacc (not plain Bass): its compile() splits multi-sem sync waits into
    # event semaphores (walrus only encodes 1 wait per instruction) and
    # auto-inserts GPSIMD library / ACT table loads.
    nc = _Bacc("TRN2")

    xt_d = nc.declare_dram_parameter("xt", [NCH, P, SEQ], BF16, isOutput=False)
    ct_d = nc.declare_dram_parameter("ct", [NCH, P, SEQ], BF16, isOutput=False)
    wq_d = nc.declare_dram_parameter("wq", [NCH, P, DIM], BF16, isOutput=False)
    wk_d = nc.declare_dram_parameter("wk", [NCH, P, DIM], BF16, isOutput=False)
    wv_d = nc.declare_dram_parameter("wv", [NCH, P, DIM], BF16, isOutput=False)
    wo_d = nc.declare_dram_parameter("wo", [NCH, P, DIM], BF16, isOutput=False)
    bq_d = nc.declare_dram_parameter("bq", [P, NCH], F32, isOutput=False)
    bk_d = nc.declare_dram_parameter("bk", [P, NCH], F32, isOutput=False)
    bv_d = nc.declare_dram_parameter("bv", [DIM], F32, isOutput=False)
    bo_d = nc.declare_dram_parameter("bo", [DIM], F32, isOutput=False)
    # bf16 output halves the non-overlappable final DMA; host upcasts.
    out_d = nc.declare_dram_parameter("out", [SEQ, DIM], BF16, isOutput=True)

    with tile.TileContext(nc) as tc:
        with (
            tc.tile_pool(name="big", bufs=1) as big,
            tc.tile_pool(name="wts", bufs=1) as wts,
            tc.tile_pool(name="ctot", bufs=1) as ctot,
            tc.tile_pool(name="expp", bufs=4) as expp,
            tc.tile_pool(name="smal", bufs=2) as smal,
            tc.tile_pool(name="outp", bufs=3) as outp,
            tc.tile_pool(name="ppr", bufs=2, space="PSUM") as ppr,
            tc.tile_pool(name="pot", bufs=2, space="PSUM") as pot,
            tc.tile_pool(name="pst", bufs=2, space="PSUM") as pst,
        ):
            # ---- persistent SBUF tensors ----
            # wv/wo rotate through one slot: wv is dead once the V projection
            # ends, and wo is only read by the final projection.
            ct_sb = ctot.tile([P, NCH, SEQ], BF16, tag="ctot", name="ct")
            wv_sb = wts.tile([P, NCH, DIM], BF16, tag="w", name="wv")
            wk_sb = big.tile([P, NCH, DIM], BF16, tag="wk")
            wq_sb = big.tile([P, NCH, DIM], BF16, tag="wq")
            wo_sb = wts.tile([P, NCH, DIM], BF16, tag="w", name="wo")
            xt_sb = big.tile([P, NCH, SEQ], BF16, tag="xt")
            kt_sb = big.tile([P, NCH, SEQ], BF16, tag="kt")
            qt_sb = big.tile([P, NCH, SEQ], BF16, tag="qt")
            vaug_sb = big.tile([P, NCH, H * HW], BF16, tag="vaug")
            bq_sb = big.tile([P, NCH], F32, tag="bq")
            bk_sb = big.tile([P, NCH], F32, tag="bk")
            bvb_sb = big.tile([P, DIM], F32, tag="bvb")
            bob_sb = big.tile([P, DIM], F32, tag="bob")

            # ---- input DMAs, ordered by first use (K projection first) ----
            for j in range(NCH):
                nc.sync.dma_start(out=ct_sb[:, j, 0:512], in_=ct_d[j][:, 0:512])
            nc.sync.dma_start(out=bk_sb, in_=bk_d[:, :])
            # wk lands in column halves: the first 4 K-projection groups
            # (mh=0, jq<4) only need ct half 0 + wk columns 0:512, so the PE
            # starts after ~2 MB of DMA instead of 3 MB.
            for ch in range(2):
                for j in range(NCH):
                    nc.sync.dma_start(
                        out=wk_sb[:, j, ch * 512:(ch + 1) * 512],
                        in_=wk_d[j][:, ch * 512:(ch + 1) * 512])
            for j in range(NCH):
                nc.sync.dma_start(out=ct_sb[:, j, 512:1024], in_=ct_d[j][:, 512:1024])
            for j in range(NCH):
                nc.sync.dma_start(out=xt_sb[:, j, :], in_=xt_d[j])
            for j in range(NCH):
                nc.sync.dma_start(out=wq_sb[:, j, :], in_=wq_d[j])
            nc.sync.dma_start(out=bq_sb, in_=bq_d[:, :])
            for j in range(NCH):
                nc.sync.dma_start(out=wv_sb[:, j, :], in_=wv_d[j])
            for (dst, src) in ((bvb_sb, bv_d), (bob_sb, bo_d)):
                ap = src[:]
                bcast = bass.AP(tensor=ap.tensor, offset=ap.offset,
                                ap=[[0, P]] + ap.ap)
                nc.sync.dma_start(out=dst, in_=bcast)
            # wo reuses wv's slot: its DMA waits until the V phase finishes;
            # wo itself is only read by the final projection, much later.
            for j in range(NCH):
                nc.sync.dma_start(out=wo_sb[:, j, :], in_=wo_d[j])

            vaug4 = vaug_sb.rearrange("p j (h e) -> p j h e", e=HW)
            nc.vector.memset(vaug4[:, :, :, HD:HW], 1.0)
            ones_sb = big.tile([1, HD], BF16, tag="ones")
            nc.vector.memset(ones_sb, 1.0)

            # ---- V = c Wv + bv, written per-head into the augmented layout.
            # Emitted in dh halves: heads 0-7 (dh=0) are needed by the first
            # attention pair, heads 8-15 (dh=1) only from pair 4 on, so the
            # halves are slotted around pair 0 to start ACT early. ----
            def v_phase(dh):
                for mm in range(NCH):
                    pv = ppr.tile([P, 512], F32, tag="ppr", name="pv")
                    for k in range(NCH):
                        nc.tensor.matmul(
                            pv,
                            lhsT=ct_sb[:, k, mm * P:(mm + 1) * P],
                            rhs=wv_sb[:, k, dh * 512:(dh + 1) * 512],
                            start=(k == 0), stop=(k == NCH - 1),
                        )
                    pvv = pv.rearrange("p (h e) -> p h e", e=HD)
                    bvv = bvb_sb[:, dh * 512:(dh + 1) * 512].rearrange(
                        "p (h e) -> p h e", e=HD)
                    nc.vector.tensor_add(
                        vaug4[:, mm, dh * 8:(dh + 1) * 8, 0:HD], pvv, bvv)

            def q_proj(jq):
                for mh in range(2):
                    pq = ppr.tile([P, 512], F32, tag="ppr", name="pq")
                    for k in range(NCH):
                        nc.tensor.matmul(
                            pq,
                            lhsT=wq_sb[:, k, jq * P:(jq + 1) * P],
                            rhs=xt_sb[:, k, mh * 512:(mh + 1) * 512],
                            start=(k == 0), stop=(k == NCH - 1),
                        )
                    nc.vector.tensor_scalar_add(
                        qt_sb[:, jq, mh * 512:(mh + 1) * 512], pq,
                        bq_sb[:, jq:jq + 1])

            def k_proj(jq, mh):
                pk = ppr.tile([P, 512], F32, tag="ppr", name="pk")
                for k in range(NCH):
                    nc.tensor.matmul(
                        pk,
                        lhsT=wk_sb[:, k, jq * P:(jq + 1) * P],
                        rhs=ct_sb[:, k, mh * 512:(mh + 1) * 512],
                        start=(k == 0), stop=(k == NCH - 1),
                    )
                nc.vector.tensor_scalar_add(
                    kt_sb[:, jq, mh * 512:(mh + 1) * 512], pk,
                    bk_sb[:, jq:jq + 1])

            # ---- fused attention ----
            ot_sb = ctot.tile([P, NCH, SEQ], BF16, tag="ctot", name="ot")

            # Deferred normalize tail: a block's ln/exp + PE broadcast +
            # normalize multiply are emitted only after the NEXT block's
            # matmuls, so the ACT latency never head-of-line blocks the PE.
            pend = []

            def flush_tail():
                for (cs_, otsA_, otsB_, jh_, nsl_) in pend:
                    # 1/colsum as exp(-ln(colsum)) on ACT (Ln and Exp share
                    # one table set).  DVE reciprocal was tried here and is
                    # far slower (multi-pass Newton on a 1-partition tile).
                    rl = smal.tile([1, 2, 512], F32, tag="rl", name="rl")
                    nc.scalar.activation(rl, cs_, LOG)
                    rc = smal.tile([1, 2, 512], BF16, tag="rc", name="rc")
                    nc.scalar.activation(rc, rl, EXP, scale=-1.0)
                    # DVE may read only one PSUM operand, so the P@V rows
                    # were copied to SBUF (ots) at block time; rbp stays in
                    # PSUM.
                    rbp = ppr.tile([P, 512], F32, tag="ppr", name="rbp")
                    nc.tensor.matmul(rbp[0:HD, :], lhsT=ones_sb,
                                     rhs=rc[:, 0, :], start=True, stop=True,
                                     tile_position=(0, 0))
                    nc.tensor.matmul(rbp[HD:P, :], lhsT=ones_sb,
                                     rhs=rc[:, 1, :], start=True, stop=True,
                                     tile_position=(0, 64))
                    nc.vector.tensor_mul(
                        ot_sb[0:HD, jh_, nsl_], otsA_, rbp[0:HD, :])
                    nc.vector.tensor_mul(
                        ot_sb[HD:P, jh_, nsl_], otsB_, rbp[HD:P, :])
                pend.clear()

            def st_block(jh, nh):
                nsl = slice(nh * 512, (nh + 1) * 512)
                exA = expp.tile([P, NCH, 512], BF16, tag="ex", name="exA")
                exB = expp.tile([P, NCH, 512], BF16, tag="ex", name="exB")
                for mg in range(4):
                    psA = pst.tile([P, 2, 512], F32, tag="pst", name="psA")
                    psB = pst.tile([P, 2, 512], F32, tag="pst", name="psB")
                    for u in range(2):
                        mm = 2 * mg + u
                        msl = slice(mm * P, (mm + 1) * P)
                        nc.tensor.matmul(
                            psA[:, u, :],
                            lhsT=kt_sb[0:HD, jh, msl],
                            rhs=qt_sb[0:HD, jh, nsl],
                            start=True, stop=True,
                            tile_position=(0, 0),
                        )
                        nc.tensor.matmul(
                            psB[:, u, :],
                            lhsT=kt_sb[HD:P, jh, msl],
                            rhs=qt_sb[HD:P, jh, nsl],
                            start=True, stop=True,
                            tile_position=(64, 0),
                        )
                    nc.scalar.activation(exA[:, 2 * mg:2 * mg + 2, :], psA, EXP)
                    nc.scalar.activation(exB[:, 2 * mg:2 * mg + 2, :], psB, EXP)
                return exA, exB, nsl

            def pv_block(jh, blk):
                exA, exB, nsl = blk
                poA = pot.tile([HD + 1, 512], F32, tag="pot", name="poA")
                for mm in range(NCH):
                    nc.tensor.matmul(
                        poA,
                        lhsT=vaug_sb[:, mm, (2 * jh) * HW:(2 * jh + 1) * HW],
                        rhs=exA[:, mm, :],
                        start=(mm == 0), stop=(mm == NCH - 1),
                    )
                poB = pot.tile([HD + 1, 512], F32, tag="pot", name="poB")
                for mm in range(NCH):
                    nc.tensor.matmul(
                        poB,
                        lhsT=vaug_sb[:, mm, (2 * jh + 1) * HW:(2 * jh + 2) * HW],
                        rhs=exB[:, mm, :],
                        start=(mm == 0), stop=(mm == NCH - 1),
                    )
                cs = smal.tile([1, 2, 512], F32, tag="cs", name="cs")
                nc.vector.tensor_copy(cs[:, 0, :], poA[HD:HD + 1, :])
                nc.vector.tensor_copy(cs[:, 1, :], poB[HD:HD + 1, :])
                otsA = smal.tile([HD, 512], BF16, tag="otsA", name="otsA")
                nc.vector.tensor_copy(otsA, poA[0:HD, :])
                otsB = smal.tile([HD, 512], BF16, tag="otsB", name="otsB")
                nc.vector.tensor_copy(otsB, poB[0:HD, :])
                flush_tail()
                pend.append((cs, otsA, otsB, jh, nsl))

            # ---- out = O Wo + bo (emitted in nn-ranges so the first half
            # can overlap the last attention block's softmax tail) ----
            def out_proj(nn0, nn1):
                for nn in range(nn0, nn1):
                    for dh in range(2):
                        pf = ppr.tile([P, 512], F32, tag="ppr", name="pf")
                        for j in range(NCH):
                            nc.tensor.matmul(
                                pf,
                                lhsT=ot_sb[:, j, nn * P:(nn + 1) * P],
                                rhs=wo_sb[:, j, dh * 512:(dh + 1) * 512],
                                start=(j == 0), stop=(j == NCH - 1),
                            )
                        of = outp.tile([P, 512], BF16, tag="of", name="of")
                        nc.vector.tensor_add(
                            of, pf, bob_sb[:, dh * 512:(dh + 1) * 512])
                        nc.sync.dma_start(
                            out=out_d[nn * P:(nn + 1) * P,
                                      dh * 512:(dh + 1) * 512],
                            in_=of)

            for mh in range(2):
                for j_k in range(NCH):
                    k_proj(j_k, mh)
            q_proj(0)
            blk = st_block(0, 0)
            v_phase(0)
            pv_block(0, blk)
            blk = st_block(0, 1)
            v_phase(1)
            pv_block(0, blk)
            q_proj(1)
            for jh in range(1, NCH):
                if jh + 1 < NCH:
                    q_proj(jh + 1)
                for nh in range(2):
                    pv_block(jh, st_block(jh, nh))
            # pv_block(7, nh=1) already flushed the (7, nh=0) tail, so all
            # n<512 rows of ot are complete: project+store them while the
            # (7, nh=1) softmax tail drains, then flush it and do the rest.
            out_proj(0, 4)
            flush_tail()
            out_proj(4, NCH)

    nc.compile()
    return nc


_STATE: dict = {}
LAST_EXEC_NS = None
LAST_PROFILE = None


def _prep_in_maps(x, context, Wq, bq, Wk, bk, Wv, bv, Wo, bo):
    def wpack(w, scale=1.0):
        return (np.asarray(w, np.float32) * scale).astype(NPBF16).reshape(
            NCH, P, DIM)

    wq_r = wpack(Wq, SCALE)
    wk_r = wpack(Wk)
    wv_r = wpack(Wv)
    wo_r = wpack(Wo)
    bq_r = np.ascontiguousarray(
        (np.asarray(bq, np.float32) * SCALE).reshape(NCH, P).T)
    bk_r = np.ascontiguousarray(np.asarray(bk, np.float32).reshape(NCH, P).T)
    bv_r = np.asarray(bv, np.float32)
    bo_r = np.asarray(bo, np.float32)

    in_maps = []
    for c in range(B):
        xt_c = np.ascontiguousarray(np.asarray(x[c], np.float32).T).astype(
            NPBF16).reshape(NCH, P, SEQ)
        ct_c = np.ascontiguousarray(np.asarray(context[c], np.float32).T).astype(
            NPBF16).reshape(NCH, P, SEQ)
        in_maps.append({
            "xt": xt_c, "ct": ct_c,
            "wq": wq_r, "wk": wk_r, "wv": wv_r, "wo": wo_r,
            "bq": bq_r, "bk": bk_r, "bv": bv_r, "bo": bo_r,
        })
    return in_maps


def kernel(x, context, Wq, bq, Wk, bk, Wv, bv, Wo, bo):
    global LAST_EXEC_NS, LAST_PROFILE
    from concourse.bass_utils import run_bass_kernel_spmd

    if "nc" not in _STATE:
        _STATE["nc"] = build_nc()
    nc = _STATE["nc"]

    in_maps = _prep_in_maps(x, context, Wq, bq, Wk, bk, Wv, bv, Wo, bo)
    trace = bool(int(os.environ.get("KERNEL_TRACE", "0")))
    kw = {}
    tmpdir = os.environ.get("KERNEL_TMPDIR")
    if tmpdir:
        os.makedirs(tmpdir, exist_ok=True)
        kw["tmpdir"] = tmpdir
    res = run_bass_kernel_spmd(nc, in_maps, list(range(B)), trace=trace, **kw)
    LAST_EXEC_NS = res.exec_time_ns
    LAST_PROFILE = res.profile_json
    out = np.stack([res.results[c]["out"] for c in range(B)], axis=0)
    return out.astype(np.float32)


# revision 28
# speedup vs baseline: 1.0754x; 1.0449x over previous
"""Cross-attention kernel for Trainium2, batch-data-parallel over 8 NeuronCores.

Reference computation (per batch element b):
    q = x Wq + bq ; k = c Wk + bk ; v = c Wv + bv          (DIM=1024)
    per head h (16 heads, d=64):
        S = (q_h k_h^T) * d^-0.5 ; P = softmax(S, axis=-1) ; o_h = P v_h
    out = concat_h(o_h) Wo + bo

Layout strategy (per core, one batch element):
    Host passes x^T and c^T (bf16) so every matmul contraction dim sits on
    SBUF partitions.  QT=[dout,n], KT=[dout,m] are produced directly by
    lhsT=W (stationary), rhs=x^T.  Attention scores are computed transposed,
    ST=[m,n], which makes P@V a plain accumulation with stationary V[m,d].
    V is augmented with a ones column so the softmax denominator falls out of
    the same matmul (row d of the PSUM tile).  Softmax skips the max-subtract
    (scores are ~N(0,1), exp cannot overflow).  The final projection consumes
    OT=[hd,n] as the stationary operand, yielding out=[n,dout] directly.

    The two heads of a pair occupy disjoint 64-partition halves, so their
    score matmuls (K=64) are issued as an explicit 64x128 row-tiled pair
    (tile_position (0,0)/(64,0)) for PE subarray concurrency.  The softmax
    tail (ln/exp reciprocal on ACT, PE broadcast, normalize multiply) is
    deferred one block so it never head-of-line blocks the PE.  The final
    projection for n<512 is emitted before the last block's tail flush so it
    overlaps the tail drain; output is stored bf16 (host upcasts) to halve
    the non-overlappable final DMA.
"""

import os

import numpy as np
import ml_dtypes

import concourse.bass as bass
import concourse.bacc as bacc
import concourse.mybir as mybir
import concourse.tile as tile

B = 8
SEQ = 1024          # N == M == 1024
DIM = 1024
H = 16
HD = DIM // H       # 64
SCALE = HD ** -0.5
P = 128
NCH = DIM // P      # 8
HW = HD + 1         # head width in the augmented V (64 values + ones col)

BF16 = mybir.dt.bfloat16
F32 = mybir.dt.float32
NPBF16 = ml_dtypes.bfloat16
EXP = mybir.ActivationFunctionType.Exp
LOG = mybir.ActivationFunctionType.Ln


class _Bacc(bacc.Bacc):
    def insert_act_table_loads(self):
        # Prefer natural_log_exp_and_others (has BOTH Exp and Ln) so the
        # softmax exp and the exp(-ln) reciprocal share one table set —
        # otherwise the pass alternates sets and pays ~2.7us per switch.
        from concourse.hw_specs import get_activation_tables
        import bass_rust as _br
        tables = list(get_activation_tables(self.m.arch).items())
        canon = [k for k, _ in tables]
        tables.sort(key=lambda kv: kv[0] != "natural_log_exp_and_others")
        _br.insert_act_table_loads(self, tables)
        # The rust pass numbers sets by position in the list it was given;
        # walrus maps act_func_set_id against act_info.json's canonical
        # order.  Re-point every emitted load (all natural_log_exp here) at
        # the canonical index.
        want = canon.index("natural_log_exp_and_others")
        for f in self.m.functions:
            for b in f.blocks:
                for i in b.instructions:
                    if isinstance(i, mybir.InstLoadActFuncSet):
                        i.act_func_set_id = want


def build_nc() -> bass.Bass:
    # Bacc (not plain Bass): its compile() splits multi-sem sync waits into
    # event semaphores (walrus only encodes 1 wait per instruction) and
    # auto-inserts GPSIMD library / ACT table loads.
    nc = _Bacc("TRN2")

    xt_d = nc.declare_dram_parameter("xt", [NCH, P, SEQ], BF16, isOutput=False)
    ct_d = nc.declare_dram_parameter("ct", [NCH, P, SEQ], BF16, isOutput=False)
    wq_d = nc.declare_dram_parameter("wq", [NCH, P, DIM], BF16, isOutput=False)
    wk_d = nc.declare_dram_parameter("wk", [NCH, P, DIM], BF16, isOutput=False)
    wv_d = nc.declare_dram_parameter("wv", [NCH, P, DIM], BF16, isOutput=False)
    wo_d = nc.declare_dram_parameter("wo", [NCH, P, DIM], BF16, isOutput=False)
    bq_d = nc.declare_dram_parameter("bq", [P, NCH], F32, isOutput=False)
    bk_d = nc.declare_dram_parameter("bk", [P, NCH], F32, isOutput=False)
    bv_d = nc.declare_dram_parameter("bv", [DIM], F32, isOutput=False)
    bo_d = nc.declare_dram_parameter("bo", [DIM], F32, isOutput=False)
    # bf16 output halves the non-overlappable final DMA; host upcasts.
    out_d = nc.declare_dram_parameter("out", [SEQ, DIM], BF16, isOutput=True)

    with tile.TileContext(nc) as tc:
        with (
            tc.tile_pool(name="big", bufs=1) as big,
            tc.tile_pool(name="wts", bufs=1) as wts,
            tc.tile_pool(name="ctot", bufs=1) as ctot,
            tc.tile_pool(name="expp", bufs=3) as expp,
            tc.tile_pool(name="smal", bufs=2) as smal,
            tc.tile_pool(name="outp", bufs=3) as outp,
            tc.tile_pool(name="ppr", bufs=2, space="PSUM") as ppr,
            tc.tile_pool(name="pot", bufs=2, space="PSUM") as pot,
            tc.tile_pool(name="pst", bufs=2, space="PSUM") as pst,
        ):
            # ---- persistent SBUF tensors ----
            # wv/wo rotate through one slot: wv is dead once the V projection
            # ends, and wo is only read by the final projection.
            ct_sb = ctot.tile([P, NCH, SEQ], BF16, tag="ctot", name="ct")
            wv_sb = wts.tile([P, NCH, DIM], BF16, tag="w", name="wv")
            wk_sb = big.tile([P, NCH, DIM], BF16, tag="wk")
            wq_sb = big.tile([P, NCH, DIM], BF16, tag="wq")
            wo_sb = wts.tile([P, NCH, DIM], BF16, tag="w", name="wo")
            xt_sb = big.tile([P, NCH, SEQ], BF16, tag="xt")
            kt_sb = big.tile([P, NCH, SEQ], BF16, tag="kt")
            qt_sb = big.tile([P, NCH, SEQ], BF16, tag="qt")
            vaug_sb = big.tile([P, NCH, H * HW], BF16, tag="vaug")
            bq_sb = big.tile([P, NCH], F32, tag="bq")
            bk_sb = big.tile([P, NCH], F32, tag="bk")
            bvb_sb = big.tile([P, DIM], F32, tag="bvb")
            bob_sb = big.tile([P, DIM], F32, tag="bob")

            # ---- input DMAs, ordered by first use (K projection first) ----
            for j in range(NCH):
                nc.sync.dma_start(out=ct_sb[:, j, 0:512], in_=ct_d[j][:, 0:512])
            nc.sync.dma_start(out=bk_sb, in_=bk_d[:, :])
            # wk lands in column halves: the first 4 K-projection groups
            # (mh=0, jq<4) only need ct half 0 + wk columns 0:512, so the PE
            # starts after ~2 MB of DMA instead of 3 MB.
            for ch in range(2):
                for j in range(NCH):
                    nc.sync.dma_start(
                        out=wk_sb[:, j, ch * 512:(ch + 1) * 512],
                        in_=wk_d[j][:, ch * 512:(ch + 1) * 512])
            for j in range(NCH):
                nc.sync.dma_start(out=ct_sb[:, j, 512:1024], in_=ct_d[j][:, 512:1024])
            for j in range(NCH):
                nc.sync.dma_start(out=xt_sb[:, j, :], in_=xt_d[j])
            for j in range(NCH):
                nc.sync.dma_start(out=wq_sb[:, j, :], in_=wq_d[j])
            nc.sync.dma_start(out=bq_sb, in_=bq_d[:, :])
            for j in range(NCH):
                nc.sync.dma_start(out=wv_sb[:, j, :], in_=wv_d[j])
            for (dst, src) in ((bvb_sb, bv_d), (bob_sb, bo_d)):
                ap = src[:]
                bcast = bass.AP(tensor=ap.tensor, offset=ap.offset,
                                ap=[[0, P]] + ap.ap)
                nc.sync.dma_start(out=dst, in_=bcast)
            # wo reuses wv's slot: its DMA waits until the V phase finishes;
            # wo itself is only read by the final projection, much later.
            for j in range(NCH):
                nc.sync.dma_start(out=wo_sb[:, j, :], in_=wo_d[j])

            vaug4 = vaug_sb.rearrange("p j (h e) -> p j h e", e=HW)
            nc.vector.memset(vaug4[:, :, :, HD:HW], 1.0)
            ones_sb = big.tile([1, HD], BF16, tag="ones")
            nc.vector.memset(ones_sb, 1.0)

            # ---- V = c Wv + bv, written per-head into the augmented layout.
            # Emitted in dh halves: heads 0-7 (dh=0) are needed by the first
            # attention pair, heads 8-15 (dh=1) only from pair 4 on, so the
            # halves are slotted around pair 0 to start ACT early. ----
            def v_phase(dh):
                for mm in range(NCH):
                    pv = ppr.tile([P, 512], F32, tag="ppr", name="pv")
                    for k in range(NCH):
                        nc.tensor.matmul(
                            pv,
                            lhsT=ct_sb[:, k, mm * P:(mm + 1) * P],
                            rhs=wv_sb[:, k, dh * 512:(dh + 1) * 512],
                            start=(k == 0), stop=(k == NCH - 1),
                        )
                    pvv = pv.rearrange("p (h e) -> p h e", e=HD)
                    bvv = bvb_sb[:, dh * 512:(dh + 1) * 512].rearrange(
                        "p (h e) -> p h e", e=HD)
                    nc.vector.tensor_add(
                        vaug4[:, mm, dh * 8:(dh + 1) * 8, 0:HD], pvv, bvv)

            def q_proj(jq):
                for mh in range(2):
                    pq = ppr.tile([P, 512], F32, tag="ppr", name="pq")
                    for k in range(NCH):
                        nc.tensor.matmul(
                            pq,
                            lhsT=wq_sb[:, k, jq * P:(jq + 1) * P],
                            rhs=xt_sb[:, k, mh * 512:(mh + 1) * 512],
                            start=(k == 0), stop=(k == NCH - 1),
                        )
                    nc.vector.tensor_scalar_add(
                        qt_sb[:, jq, mh * 512:(mh + 1) * 512], pq,
                        bq_sb[:, jq:jq + 1])

            def k_proj(jq, mh):
                pk = ppr.tile([P, 512], F32, tag="ppr", name="pk")
                for k in range(NCH):
                    nc.tensor.matmul(
                        pk,
                        lhsT=wk_sb[:, k, jq * P:(jq + 1) * P],
                        rhs=ct_sb[:, k, mh * 512:(mh + 1) * 512],
                        start=(k == 0), stop=(k == NCH - 1),
                    )
                nc.vector.tensor_scalar_add(
                    kt_sb[:, jq, mh * 512:(mh + 1) * 512], pk,
                    bk_sb[:, jq:jq + 1])

            # ---- fused attention ----
            ot_sb = ctot.tile([P, NCH, SEQ], BF16, tag="ctot", name="ot")

            # Deferred normalize tail: a block's ln/exp + PE broadcast +
            # normalize multiply are emitted only after the NEXT block's
            # matmuls, so the ACT latency never head-of-line blocks the PE.
            pend = []

            def flush_tail():
                for (cs_, otsA_, otsB_, jh_, nsl_) in pend:
                    # 1/colsum as exp(-ln(colsum)) on ACT (Ln and Exp share
                    # one table set).  DVE reciprocal was tried here and is
                    # far slower (multi-pass Newton on a 1-partition tile).
                    rl = smal.tile([1, 2, 512], F32, tag="rl", name="rl")
                    nc.scalar.activation(rl, cs_, LOG)
                    rc = smal.tile([1, 2, 512], BF16, tag="rc", name="rc")
                    nc.scalar.activation(rc, rl, EXP, scale=-1.0)
                    # DVE may read only one PSUM operand, so the P@V rows
                    # were copied to SBUF (ots) at block time; rbp stays in
                    # PSUM.
                    rbp = ppr.tile([P, 512], F32, tag="ppr", name="rbp")
                    nc.tensor.matmul(rbp[0:HD, :], lhsT=ones_sb,
                                     rhs=rc[:, 0, :], start=True, stop=True,
                                     tile_position=(0, 0))
                    nc.tensor.matmul(rbp[HD:P, :], lhsT=ones_sb,
                                     rhs=rc[:, 1, :], start=True, stop=True,
                                     tile_position=(0, 64))
                    nc.vector.tensor_mul(
                        ot_sb[0:HD, jh_, nsl_], otsA_, rbp[0:HD, :])
                    nc.vector.tensor_mul(
                        ot_sb[HD:P, jh_, nsl_], otsB_, rbp[HD:P, :])
                pend.clear()

            def st_block(jh, nh):
                nsl = slice(nh * 512, (nh + 1) * 512)
                exA = expp.tile([P, NCH, 512], BF16, tag="ex", name="exA")
                exB = expp.tile([P, NCH, 512], BF16, tag="ex", name="exB")
                for mg in range(4):
                    psA = pst.tile([P, 2, 512], F32, tag="pst", name="psA")
                    psB = pst.tile([P, 2, 512], F32, tag="pst", name="psB")
                    for u in range(2):
                        mm = 2 * mg + u
                        msl = slice(mm * P, (mm + 1) * P)
                        nc.tensor.matmul(
                            psA[:, u, :],
                            lhsT=kt_sb[0:HD, jh, msl],
                            rhs=qt_sb[0:HD, jh, nsl],
                            start=True, stop=True,
                            tile_position=(0, 0),
                        )
                        nc.tensor.matmul(
                            psB[:, u, :],
                            lhsT=kt_sb[HD:P, jh, msl],
                            rhs=qt_sb[HD:P, jh, nsl],
                            start=True, stop=True,
                            tile_position=(64, 0),
                        )
                    nc.scalar.activation(exA[:, 2 * mg:2 * mg + 2, :], psA, EXP)
                    nc.scalar.activation(exB[:, 2 * mg:2 * mg + 2, :], psB, EXP)
                return exA, exB, nsl

            def pv_block(jh, blk):
                exA, exB, nsl = blk
                poA = pot.tile([HD + 1, 512], F32, tag="pot", name="poA")
                for mm in range(NCH):
                    nc.tensor.matmul(
                        poA,
                        lhsT=vaug_sb[:, mm, (2 * jh) * HW:(2 * jh + 1) * HW],
                        rhs=exA[:, mm, :],
                        start=(mm == 0), stop=(mm == NCH - 1),
                    )
                poB = pot.tile([HD + 1, 512], F32, tag="pot", name="poB")
                for mm in range(NCH):
                    nc.tensor.matmul(
                        poB,
                        lhsT=vaug_sb[:, mm, (2 * jh + 1) * HW:(2 * jh + 2) * HW],
                        rhs=exB[:, mm, :],
                        start=(mm == 0), stop=(mm == NCH - 1),
                    )
                cs = smal.tile([1, 2, 512], F32, tag="cs", name="cs")
                nc.vector.tensor_copy(cs[:, 0, :], poA[HD:HD + 1, :])
                nc.vector.tensor_copy(cs[:, 1, :], poB[HD:HD + 1, :])
                otsA = smal.tile([HD, 512], BF16, tag="otsA", name="otsA")
                nc.vector.tensor_copy(otsA, poA[0:HD, :])
                otsB = smal.tile([HD, 512], BF16, tag="otsB", name="otsB")
                nc.vector.tensor_copy(otsB, poB[0:HD, :])
                flush_tail()
                pend.append((cs, otsA, otsB, jh, nsl))

            # ---- out = O Wo + bo (emitted in nn-ranges so the first half
            # can overlap the last attention block's softmax tail) ----
            def out_proj(nn0, nn1):
                for nn in range(nn0, nn1):
                    for dh in range(2):
                        pf = ppr.tile([P, 512], F32, tag="ppr", name="pf")
                        for j in range(NCH):
                            nc.tensor.matmul(
                                pf,
                                lhsT=ot_sb[:, j, nn * P:(nn + 1) * P],
                                rhs=wo_sb[:, j, dh * 512:(dh + 1) * 512],
                                start=(j == 0), stop=(j == NCH - 1),
                            )
                        of = outp.tile([P, 512], BF16, tag="of", name="of")
                        nc.vector.tensor_add(
                            of, pf, bob_sb[:, dh * 512:(dh + 1) * 512])
                        nc.sync.dma_start(
                            out=out_d[nn * P:(nn + 1) * P,
                                      dh * 512:(dh + 1) * 512],
                            in_=of)

            for mh in range(2):
                for j_k in range(NCH):
                    k_proj(j_k, mh)
            q_proj(0)
            blk = st_block(0, 0)
            v_phase(0)
            pv_block(0, blk)
            blk = st_block(0, 1)
            v_phase(1)
            pv_block(0, blk)
            q_proj(1)
            for jh in range(1, NCH):
                if jh + 1 < NCH:
                    q_proj(jh + 1)
                for nh in range(2):
                    pv_block(jh, st_block(jh, nh))
            # pv_block(7, nh=1) already flushed the (7, nh=0) tail, so all
            # n<512 rows of ot are complete: project+store them while the
            # (7, nh=1) softmax tail drains, then flush it and do the rest.
            out_proj(0, 4)
            flush_tail()
            out_proj(4, NCH)

    nc.compile()
    return nc


_STATE: dict = {}
LAST_EXEC_NS = None
LAST_PROFILE = None


def _prep_in_maps(x, context, Wq, bq, Wk, bk, Wv, bv, Wo, bo):
    def wpack(w, scale=1.0):
        return (np.asarray(w, np.float32) * scale).astype(NPBF16).reshape(
            NCH, P, DIM)

    wq_r = wpack(Wq, SCALE)
    wk_r = wpack(Wk)
    wv_r = wpack(Wv)
    wo_r = wpack(Wo)
    bq_r = np.ascontiguousarray(
        (np.asarray(bq, np.float32) * SCALE).reshape(NCH, P).T)
    bk_r = np.ascontiguousarray(np.asarray(bk, np.float32).reshape(NCH, P).T)
    bv_r = np.asarray(bv, np.float32)
    bo_r = np.asarray(bo, np.float32)

    in_maps = []
    for c in range(B):
        xt_c = np.ascontiguousarray(np.asarray(x[c], np.float32).T).astype(
            NPBF16).reshape(NCH, P, SEQ)
        ct_c = np.ascontiguousarray(np.asarray(context[c], np.float32).T).astype(
            NPBF16).reshape(NCH, P, SEQ)
        in_maps.append({
            "xt": xt_c, "ct": ct_c,
            "wq": wq_r, "wk": wk_r, "wv": wv_r, "wo": wo_r,
            "bq": bq_r, "bk": bk_r, "bv": bv_r, "bo": bo_r,
        })
    return in_maps


def kernel(x, context, Wq, bq, Wk, bk, Wv, bv, Wo, bo):
    global LAST_EXEC_NS, LAST_PROFILE
    from concourse.bass_utils import run_bass_kernel_spmd

    if "nc" not in _STATE:
        _STATE["nc"] = build_nc()
    nc = _STATE["nc"]

    in_maps = _prep_in_maps(x, context, Wq, bq, Wk, bk, Wv, bv, Wo, bo)
    trace = bool(int(os.environ.get("KERNEL_TRACE", "0")))
    kw = {}
    tmpdir = os.environ.get("KERNEL_TMPDIR")
    if tmpdir:
        os.makedirs(tmpdir, exist_ok=True)
        kw["tmpdir"] = tmpdir
    res = run_bass_kernel_spmd(nc, in_maps, list(range(B)), trace=trace, **kw)
    LAST_EXEC_NS = res.exec_time_ns
    LAST_PROFILE = res.profile_json
    out = np.stack([res.results[c]["out"] for c in range(B)], axis=0)
    return out.astype(np.float32)
